# revision 1
# baseline (speedup 1.0000x reference)
"""Bilinear (outer-product) pooling + signed-sqrt + L2-norm + skinny classifier.

Reference computes, for feat [B, D], W [C, D*D], b [C]:
    x[b, i*D+j] = feat[b,i] * feat[b,j]
    y = sign(x) * sqrt(|x| + EPS_SQRT)
    out = (y / max(||y||_2, EPS_NORM)) @ W.T + b

Identities (exact up to the EPS_SQRT inside the element sqrt, whose effect
on the output is ~1e-5 relative):
    y[b, i*D+j] ~= g[b,i] * g[b,j],   g = sign(feat) * sqrt(|feat|)
    ||y||_2^2    = (sum_i |feat[b,i]|)^2 + EPS_SQRT * D^2          (exact)
so with M_c = W[c].reshape(D, D):
    out[b,c] = g_b^T M_c g_b / norm_b + bias_c

Since only the symmetric part of M_c matters, stream just the upper
triangle of A_c = M_c + M_c^T in 128x128 blocks (diag blocks: M_c as-is)
-> 136 blocks = 17 per core across 8 cores, 0.53x the W traffic, cast to
bf16 on host (memory-bound problem; measured output rel err ~3e-3).

Per core, per class c (SPMD-uniform; all core variation is in the packed
data, not the program):
    slot s (one W-stationary matmul, FWL):
        ps[j, s*32+b] = sum_i A_s[i,j] * g[b, 128*bi_s + i]
    DVE:  V = ps * g_bj   (bf16)
    ones-matmul partition-reduce: ps2[0, (s,b)] = sum_j V[j, s, b]
    ACT copies ps2 into an output row buffer.
Host: out[b,c] = (sum_cores sum_slots ps2) / norm_b + bias_c.
"""

import sys

import numpy as np

if "/opt/trn_rl_repo" not in sys.path:
    sys.path.insert(0, "/opt/trn_rl_repo")

import ml_dtypes

import concourse.bass as bass
import concourse.bacc as bacc
import concourse.mybir as mybir
import concourse.tile as tile
from concourse.bass_utils import run_bass_kernel_spmd

B, D, C = 32, 2048, 30
EPS_SQRT = 1e-10
EPS_NORM = 1e-12

N_CORES = 8
P = 128
NB = D // P                              # 16 row/col blocks
NS = (NB * (NB + 1) // 2) // N_CORES     # 17 slots per core
UPPER = [(bi, bj) for bi in range(NB) for bj in range(bi, NB)]
assert len(UPPER) == NS * N_CORES
CPAIR = C // 2                           # W DMAs batched 2 classes at a time

_CACHE = {}


def _build_bass(repeat=1):
    nc = bacc.Bacc(None, target_bir_lowering=False, debug=False)
    w_d = nc.dram_tensor("w", [CPAIR, P, 2 * NS * P], mybir.dt.bfloat16,
                         kind="ExternalInput")
    gt_d = nc.dram_tensor("gt", [P, NS * B], mybir.dt.bfloat16, kind="ExternalInput")
    gc_d = nc.dram_tensor("gc", [P, NS * B], mybir.dt.float32, kind="ExternalInput")
    out_d = nc.dram_tensor("out", [1, C * NS * B], mybir.dt.float32,
                           kind="ExternalOutput")

    with tile.TileContext(nc) as tc:
        with (
            tc.tile_pool(name="wpool", bufs=5) as wpool,
            tc.tile_pool(name="const", bufs=1) as cpool,
            tc.tile_pool(name="spool", bufs=3) as spool,
            tc.tile_pool(name="psA", bufs=2, space=bass.MemorySpace.PSUM) as ppoolA,
            tc.tile_pool(name="psB", bufs=2, space=bass.MemorySpace.PSUM) as ppoolB,
        ):
            # consts ride the ACT HWDGE queue so they overlap the first W
            # transfer on the sync queue
            gt_sb = cpool.tile([P, NS * B], mybir.dt.bfloat16)
            nc.scalar.dma_start(gt_sb[:], gt_d[:])
            gc_sb = cpool.tile([P, NS * B], mybir.dt.float32)
            nc.scalar.dma_start(gc_sb[:], gc_d[:])
            ones_sb = cpool.tile([P, 1], mybir.dt.bfloat16)
            nc.vector.memset(ones_sb[:], 1.0)
            obuf = cpool.tile([1, C * NS * B], mybir.dt.float32)

            first = True
            for _ in range(repeat):
                for cp in range(CPAIR):
                    wt = wpool.tile([P, 2 * NS * P], mybir.dt.bfloat16)
                    if first:
                        # split the very first transfer so the PE starts
                        # after half a pair instead of a full one
                        nc.sync.dma_start(wt[:, :NS * P], w_d[cp, :, :NS * P])
                        nc.sync.dma_start(wt[:, NS * P:], w_d[cp, :, NS * P:])
                        first = False
                    else:
                        nc.sync.dma_start(wt[:], w_d[cp])
                    for h in range(2):
                        c = 2 * cp + h
                        wh = wt[:, h * NS * P:(h + 1) * NS * P]
                        ps = ppoolA.tile([P, NS * B], mybir.dt.float32)
                        for s in range(NS):
                            nc.tensor.matmul(
                                ps[:, s * B:(s + 1) * B],
                                wh[:, s * P:(s + 1) * P],
                                gt_sb[:, s * B:(s + 1) * B],
                                start=True, stop=True,
                            )
                        v = spool.tile([P, NS * B], mybir.dt.bfloat16)
                        nc.vector.tensor_mul(v[:], ps[:], gc_sb[:])
                        ps2 = ppoolB.tile([1, NS * B], mybir.dt.float32)
                        nc.tensor.matmul(ps2[:, 0:512], ones_sb[:], v[:, 0:512],
                                         start=True, stop=True)
                        nc.tensor.matmul(ps2[:, 512:NS * B], ones_sb[:],
                                         v[:, 512:NS * B], start=True, stop=True)
                        nc.scalar.copy(obuf[:, c * NS * B:(c + 1) * NS * B], ps2[:])
            nc.sync.dma_start(out_d[:], obuf[:])
    if not nc.is_finalized():
        nc.finalize()
    return nc


def _prep_inputs(feat, W):
    feat = np.asarray(feat, dtype=np.float32)
    W = np.asarray(W, dtype=np.float32)

    g = np.sign(feat) * np.sqrt(np.abs(feat))
    norm = np.sqrt(np.sum(np.abs(feat), axis=1, dtype=np.float64) ** 2
                   + EPS_SQRT * float(D) * float(D))
    norm = np.maximum(norm, EPS_NORM)

    W4 = W.reshape(C, NB, P, NB, P)  # [c, bi, i, bj, j]
    gT = np.ascontiguousarray(g.T)   # [D, B] fp32

    in_maps = []
    for k in range(N_CORES):
        blocks = UPPER[k::N_CORES]
        # wk[c, i, s, j] = A_c[bi_s, bj_s][i, j]
        wk = np.empty((C, P, NS, P), dtype=np.float32)
        for s, (bi, bj) in enumerate(blocks):
            blk = W4[:, bi, :, bj, :]
            if bi != bj:
                blk = blk + W4[:, bj, :, bi, :].transpose(0, 2, 1)
            wk[:, :, s, :] = blk
        wk = (wk.astype(ml_dtypes.bfloat16)
                .reshape(CPAIR, 2, P, NS * P)
                .transpose(0, 2, 1, 3))          # [cpair, i, half, s*j]
        wk = np.ascontiguousarray(wk).reshape(CPAIR, P, 2 * NS * P)
        gt = np.empty((P, NS, B), dtype=np.float32)
        gc = np.empty((P, NS, B), dtype=np.float32)
        for s, (bi, bj) in enumerate(blocks):
            gt[:, s, :] = gT[bi * P:(bi + 1) * P, :]
            gc[:, s, :] = gT[bj * P:(bj + 1) * P, :]
        in_maps.append({
            "w": wk,
            "gt": gt.reshape(P, NS * B).astype(ml_dtypes.bfloat16),
            "gc": np.ascontiguousarray(gc.reshape(P, NS * B)),
        })
    return in_maps, norm


def _run(inputs, trace=False, repeat=1):
    feat, W, b = inputs["feat"], inputs["W"], inputs["b"]
    assert feat.shape == (B, D) and W.shape == (C, D * D)

    key = ("nc", repeat)
    if key not in _CACHE:
        _CACHE[key] = _build_bass(repeat)
    nc = _CACHE[key]

    in_maps, norm = _prep_inputs(feat, W)
    res = run_bass_kernel_spmd(nc, in_maps, list(range(N_CORES)), trace=trace)
    parts = np.stack([r["out"] for r in res.results]).astype(np.float64)
    parts = parts.reshape(N_CORES, C, NS, B).sum(axis=(0, 2)).T  # [B, C]
    out = parts / norm[:, None] + np.asarray(b, dtype=np.float64)[None, :]
    return out.astype(np.float32), res


def kernel(**inputs):
    return _run(inputs)[0]



# revision 2
# speedup vs baseline: 1.0873x; 1.0873x over previous
"""Bilinear pooling + signed-sqrt + L2-norm + classifier, v4.

Math (same identity as v1): with g = sign(feat)*sqrt(|feat|),
    out[b,c] = g_b^T M_c g_b / norm_b + bias_c,   M_c = W[c].reshape(D,D)
Only the symmetric part matters, so each unordered block pair (u,v) of the
16x16 block grid is shipped once as A = M[u,v] + M[v,u]^T (diag: M[v,v]).

Design:
  * W blocks shipped as fp8 E3M4 (1 B/elem): 8.36 MB/core/pass, half of
    bf16. Scaled per (class, column-group) to absmax 14; the scale is
    divided back out on the host output, costing nothing on device.
  * Block->core assignment via a balanced tournament orientation of K16:
    column v takes in-edges from {v-1..v-7 mod 16} (+ {v-8} for v>=8)
    plus the diagonal, giving 8 columns of 9 blocks and 8 of 8. Core k
    owns columns (8+k) [9 blocks] and (k) [8 blocks] -> uniform SPMD
    program: 2 accumulation groups of 9+8 matmuls per class, with all
    per-core variation in the packed data.
  * Group blocks accumulate in PSUM over bi, so the elementwise multiply
    and ones-reduce shrink from 544 to 64 columns per class, batched 8
    classes per 2KB PSUM bank: one tensor_mul + one ones-matmul per bank.
  * W streams in 6 chunks (2/6/8/8/4/2 classes) alternating between the
    two HWDGE queues (sync/scalar): big chunks keep HBM near line rate,
    the small first chunk starts the PE early, the small last chunk
    shortens the drain tail.

Per core, per pass (measured on 8x axon trn2, For_i slope):
  DMA  8.36 MB W(fp8) + 0.26 MB gt/gc   ~24-25 us  <- bound
  PE   510 x (LDW 128col fp8 + MM N=32) ~11 us
  DVE  4 x tensor_mul [128, <=512]       ~3 us
Host: out[b,c] = sum_cores sum_grp lam[core,c,grp]*o[...] / norm_b + bias_c.
"""

import sys

import numpy as np

if "/opt/trn_rl_repo" not in sys.path:
    sys.path.insert(0, "/opt/trn_rl_repo")

import ml_dtypes

import concourse.bass as bass
import concourse.bacc as bacc
import concourse.mybir as mybir
import concourse.tile as tile
from concourse.bass_utils import run_bass_kernel_spmd

B, D, C = 32, 2048, 30
EPS_SQRT = 1e-10
EPS_NORM = 1e-12

N_CORES = 8
P = 128
NB = D // P            # 16 block-columns
NS = 17                # 9 + 8 blocks per core
SMAX = 14.0            # fp8 e3m4 absmax target (max normal 15.5)

BANKS = [(0, 8), (8, 16), (16, 24), (24, 30)]
GRPS = ((0, 9), (9, 17))
GW = 2 * B             # 64 psum cols per class (2 groups x 32)
CHUNKS = [(0, 2), (2, 8), (8, 16), (16, 24), (24, 28), (28, 30)]

_CACHE = {}


def _core_cols(k):
    """(column, [bi list]) for core k's two groups (9 then 8 blocks)."""
    vA = 8 + k
    biA = [(vA - d) % NB for d in range(1, 8)] + [vA - 8, vA]
    vB = k
    biB = [(vB - d) % NB for d in range(1, 8)] + [vB]
    return (vA, biA), (vB, biB)


def _build_bass(repeat=1, loop_n=None):
    """One SPMD pass (python-unrolled `repeat`), optionally HW-looped."""
    nc = bacc.Bacc(None, target_bir_lowering=False, debug=False)
    w_d = nc.dram_tensor("w", [P, C * NS * P], mybir.dt.float8e3,
                         kind="ExternalInput")
    # gt (17*32) and gc-rep (8*64) packed in one tensor: one DMA setup cost
    gg_d = nc.dram_tensor("gg", [P, NS * B + 8 * GW], mybir.dt.bfloat16,
                          kind="ExternalInput")
    out_d = nc.dram_tensor("out", [1, C * GW], mybir.dt.float32,
                           kind="ExternalOutput")

    with tile.TileContext(nc) as tc:
        with (
            tc.tile_pool(name="wpool", bufs=3) as wpool,
            tc.tile_pool(name="const", bufs=1) as cpool,
            tc.tile_pool(name="spool", bufs=2) as spool,
            tc.tile_pool(name="psA", bufs=2, space=bass.MemorySpace.PSUM) as ppoolA,
            tc.tile_pool(name="psB", bufs=2, space=bass.MemorySpace.PSUM) as ppoolB,
        ):
            # gg rides the scalar HWDGE queue ahead of that queue's W chunks
            gg_sb = cpool.tile([P, NS * B + 8 * GW], mybir.dt.bfloat16)
            nc.scalar.dma_start(gg_sb[:], gg_d[:])
            gt_sb = gg_sb[:, :NS * B]
            gc_sb = gg_sb[:, NS * B:]
            ones_sb = cpool.tile([P, 1], mybir.dt.bfloat16)
            nc.vector.memset(ones_sb[:], 1.0)
            obuf = cpool.tile([1, C * GW], mybir.dt.float32)

            def one_pass():
                ps = None
                for i, (cs, ce) in enumerate(CHUNKS):
                    cols = (ce - cs) * NS * P
                    off = cs * NS * P
                    wt = wpool.tile([P, cols], mybir.dt.float8e3)
                    eng = nc.sync if i % 2 == 0 else nc.scalar
                    eng.dma_start(wt[:], w_d[:, off:off + cols])
                    bank = cs // 8
                    b0, b1 = BANKS[bank]
                    ncc = b1 - b0
                    if cs == b0:  # bank starts with this chunk
                        ps = ppoolA.tile([P, ncc * GW], mybir.dt.float32)
                    for h in range(ce - cs):
                        cc = cs + h - b0
                        base = h * NS * P
                        for grp, (t0, t1) in enumerate(GRPS):
                            pcol = cc * GW + grp * B
                            for t in range(t0, t1):
                                nc.tensor.matmul(
                                    ps[:, pcol:pcol + B],
                                    wt[:, base + t * P:base + (t + 1) * P],
                                    gt_sb[:, t * B:(t + 1) * B],
                                    start=(t == t0), stop=(t == t1 - 1),
                                )
                    if ce == b1:  # bank complete
                        v = spool.tile([P, ncc * GW], mybir.dt.bfloat16)
                        nc.vector.tensor_mul(v[:], ps[:], gc_sb[:, :ncc * GW])
                        ps2 = ppoolB.tile([1, ncc * GW], mybir.dt.float32)
                        nc.tensor.matmul(ps2[:], ones_sb[:], v[:],
                                         start=True, stop=True)
                        nc.scalar.copy(obuf[:, b0 * GW:b1 * GW], ps2[:])

            if loop_n is not None:
                with tc.For_i(0, loop_n):
                    one_pass()
            else:
                for _ in range(repeat):
                    one_pass()
            nc.sync.dma_start(out_d[:], obuf[:])
    if not nc.is_finalized():
        nc.finalize()
    return nc


def _prep_inputs(feat, W):
    feat = np.asarray(feat, dtype=np.float32)
    W = np.asarray(W, dtype=np.float32)

    g = np.sign(feat) * np.sqrt(np.abs(feat))
    norm = np.sqrt(np.sum(np.abs(feat), axis=1, dtype=np.float64) ** 2
                   + EPS_SQRT * float(D) * float(D))
    norm = np.maximum(norm, EPS_NORM)

    W4 = W.reshape(C, NB, P, NB, P)  # [c, bi, i, bj, j]
    gbf = g.astype(ml_dtypes.bfloat16).astype(np.float32)
    gT = np.ascontiguousarray(gbf.T)  # [D, B]

    in_maps = []
    lams = []
    for k in range(N_CORES):
        groups = _core_cols(k)
        wk = np.empty((C, NS, P, P), dtype=np.float32)  # [c, t, i, j]
        lam = np.empty((C, 2), dtype=np.float32)
        t = 0
        for grp, (bj, bis) in enumerate(groups):
            ts = slice(t, t + len(bis))
            for dt_, bi in enumerate(bis):
                blk = W4[:, bi, :, bj, :]
                if bi != bj:
                    blk = blk + W4[:, bj, :, bi, :].transpose(0, 2, 1)
                wk[:, t + dt_] = blk
            lam[:, grp] = np.abs(wk[:, ts]).max(axis=(1, 2, 3)) / SMAX
            wk[:, ts] /= lam[:, grp][:, None, None, None]
            t += len(bis)
        lams.append(lam)
        # [c, t, i, j] -> [i, (c, t, j)]
        wk8 = (wk.transpose(2, 0, 1, 3).reshape(P, C * NS * P)
               .astype(ml_dtypes.float8_e3m4))

        gg = np.empty((P, NS * B + 8 * GW), dtype=np.float32)
        t = 0
        for bj, bis in groups:
            for bi in bis:
                gg[:, t * B:(t + 1) * B] = gT[bi * P:(bi + 1) * P]
                t += 1
        base = NS * B
        for grp, (bj, bis) in enumerate(groups):
            for cc in range(8):
                o = base + cc * GW + grp * B
                gg[:, o:o + B] = gT[bj * P:(bj + 1) * P]
        in_maps.append({
            "w": np.ascontiguousarray(wk8),
            "gg": gg.astype(ml_dtypes.bfloat16),
        })
    return in_maps, norm, np.stack(lams)  # lams [cores, C, 2]


def _run(inputs, trace=False, repeat=1):
    feat, W, b = inputs["feat"], inputs["W"], inputs["b"]
    assert feat.shape == (B, D) and W.shape == (C, D * D)

    key = ("nc", repeat)
    if key not in _CACHE:
        _CACHE[key] = _build_bass(repeat)
    nc = _CACHE[key]

    in_maps, norm, lams = _prep_inputs(feat, W)
    res = run_bass_kernel_spmd(nc, in_maps, list(range(N_CORES)), trace=trace)
    parts = np.stack([r["out"] for r in res.results]).astype(np.float64)
    parts = parts.reshape(N_CORES, C, 2, B) * lams[:, :, :, None]
    parts = parts.sum(axis=(0, 2)).T  # [B, C]
    out = parts / norm[:, None] + np.asarray(b, dtype=np.float64)[None, :]
    return out.astype(np.float32), res


def kernel(**inputs):
    return _run(inputs)[0]


# revision 3
# speedup vs baseline: 1.1691x; 1.0752x over previous
"""Bilinear pooling + signed-sqrt + L2-norm + classifier, v4.

Math (same identity as v1): with g = sign(feat)*sqrt(|feat|),
    out[b,c] = g_b^T M_c g_b / norm_b + bias_c,   M_c = W[c].reshape(D,D)
Only the symmetric part matters, so each unordered block pair (u,v) of the
16x16 block grid is shipped once as A = M[u,v] + M[v,u]^T (diag: M[v,v]).

Design:
  * W blocks shipped as fp8 E3M4 (1 B/elem): 8.36 MB/core/pass, half of
    bf16. Scaled per (class, column-group) to absmax 14; the scale is
    divided back out on the host output, costing nothing on device.
  * Block->core assignment via a balanced tournament orientation of K16:
    column v takes in-edges from {v-1..v-7 mod 16} (+ {v-8} for v>=8)
    plus the diagonal, giving 8 columns of 9 blocks and 8 of 8. Core k
    owns columns (8+k) [9 blocks] and (k) [8 blocks] -> uniform SPMD
    program: 2 accumulation groups of 9+8 matmuls per class, with all
    per-core variation in the packed data.
  * Group blocks accumulate in PSUM over bi, so the elementwise multiply
    and ones-reduce shrink from 544 to 64 columns per class, batched 8
    classes per 2KB PSUM bank: one tensor_mul + one ones-matmul per bank.
  * W streams in 6 chunks (2/6/8/8/4/2 classes) alternating between the
    two HWDGE queues (sync/scalar): big chunks keep HBM near line rate,
    the small first chunk starts the PE early, the small last chunk
    shortens the drain tail.

Per core, per pass (measured on 8x axon trn2, For_i slope):
  DMA  8.36 MB W(fp8) + 0.26 MB gt/gc   ~24-25 us  <- bound
  PE   510 x (LDW 128col fp8 + MM N=32) ~11 us
  DVE  4 x tensor_mul [128, <=512]       ~3 us
Host: out[b,c] = sum_cores sum_grp lam[core,c,grp]*o[...] / norm_b + bias_c.
"""

import sys

import numpy as np

if "/opt/trn_rl_repo" not in sys.path:
    sys.path.insert(0, "/opt/trn_rl_repo")

import ml_dtypes

import concourse.bass as bass
import concourse.bacc as bacc
import concourse.mybir as mybir
import concourse.tile as tile
from concourse.bass_utils import run_bass_kernel_spmd

B, D, C = 32, 2048, 30
EPS_SQRT = 1e-10
EPS_NORM = 1e-12

N_CORES = 8
P = 128
NB = D // P            # 16 block-columns
NS = 17                # 9 + 8 blocks per core
SMAX = 14.0            # fp8 e3m4 absmax target (max normal 15.5)

BANKS = [(0, 8), (8, 16), (16, 24), (24, 30)]
GRPS = ((0, 9), (9, 17))
GW = 2 * B             # 64 psum cols per class (2 groups x 32)
CHUNKS = [(0, 2), (2, 8), (8, 16), (16, 24), (24, 28), (28, 30)]

_CACHE = {}


def _core_cols(k):
    """(column, [bi list]) for core k's two groups (9 then 8 blocks)."""
    vA = 8 + k
    biA = [(vA - d) % NB for d in range(1, 8)] + [vA - 8, vA]
    vB = k
    biB = [(vB - d) % NB for d in range(1, 8)] + [vB]
    return (vA, biA), (vB, biB)


def _build_bass(repeat=1, loop_n=None):
    """One SPMD pass (python-unrolled `repeat`), optionally HW-looped."""
    nc = bacc.Bacc(None, target_bir_lowering=False, debug=False)
    w_d = nc.dram_tensor("w", [P, C * NS * P], mybir.dt.float8e3,
                         kind="ExternalInput")
    # gt (17*32) and gc-rep (8*64) packed in one tensor: one DMA setup cost
    gg_d = nc.dram_tensor("gg", [P, NS * B + 8 * GW], mybir.dt.bfloat16,
                          kind="ExternalInput")
    out_d = nc.dram_tensor("out", [1, C * GW], mybir.dt.float32,
                           kind="ExternalOutput")

    with tile.TileContext(nc) as tc:
        with (
            tc.tile_pool(name="wpool", bufs=4) as wpool,
            tc.tile_pool(name="const", bufs=1) as cpool,
            tc.tile_pool(name="spool", bufs=2) as spool,
            tc.tile_pool(name="psA", bufs=2, space=bass.MemorySpace.PSUM) as ppoolA,
            tc.tile_pool(name="psB", bufs=2, space=bass.MemorySpace.PSUM) as ppoolB,
        ):
            # gg rides the scalar HWDGE queue ahead of that queue's W chunks
            gg_sb = cpool.tile([P, NS * B + 8 * GW], mybir.dt.bfloat16)
            nc.scalar.dma_start(gg_sb[:], gg_d[:])
            gt_sb = gg_sb[:, :NS * B]
            gc_sb = gg_sb[:, NS * B:]
            ones_sb = cpool.tile([P, 1], mybir.dt.bfloat16)
            nc.vector.memset(ones_sb[:], 1.0)
            obuf = cpool.tile([1, C * GW], mybir.dt.float32)

            def one_pass():
                ps = None
                emitted = 0

                def emit(ps, b0, lo_c, hi_c):
                    n = hi_c - lo_c
                    lo, hi = (lo_c - b0) * GW, (hi_c - b0) * GW
                    v = spool.tile([P, n * GW], mybir.dt.bfloat16)
                    nc.vector.tensor_mul(v[:], ps[:, lo:hi], gc_sb[:, lo:hi])
                    ps2 = ppoolB.tile([1, n * GW], mybir.dt.float32)
                    nc.tensor.matmul(ps2[:], ones_sb[:], v[:],
                                     start=True, stop=True)
                    nc.scalar.copy(obuf[:, lo_c * GW:hi_c * GW], ps2[:])
                for i, (cs, ce) in enumerate(CHUNKS):
                    cols = (ce - cs) * NS * P
                    off = cs * NS * P
                    wt = wpool.tile([P, cols], mybir.dt.float8e3)
                    eng = nc.sync if i % 2 == 0 else nc.scalar
                    eng.dma_start(wt[:], w_d[:, off:off + cols])
                    bank = cs // 8
                    b0, b1 = BANKS[bank]
                    ncc = b1 - b0
                    if cs == b0:  # bank starts with this chunk
                        ps = ppoolA.tile([P, ncc * GW], mybir.dt.float32)
                    for h in range(ce - cs):
                        cc = cs + h - b0
                        base = h * NS * P
                        for grp, (t0, t1) in enumerate(GRPS):
                            pcol = cc * GW + grp * B
                            for t in range(t0, t1):
                                nc.tensor.matmul(
                                    ps[:, pcol:pcol + B],
                                    wt[:, base + t * P:base + (t + 1) * P],
                                    gt_sb[:, t * B:(t + 1) * B],
                                    start=(t == t0), stop=(t == t1 - 1),
                                )
                    # flush completed 4-class half-banks (2 at bank3 end)
                    while emitted + 4 <= min(ce, b1):
                        emit(ps, b0, emitted, emitted + 4)
                        emitted += 4
                    if ce == b1 and emitted < b1:
                        emit(ps, b0, emitted, b1)
                        emitted = b1

            if loop_n is not None:
                with tc.For_i(0, loop_n):
                    one_pass()
            else:
                for _ in range(repeat):
                    one_pass()
            nc.sync.dma_start(out_d[:], obuf[:])
    if not nc.is_finalized():
        nc.finalize()
    return nc


def _prep_inputs(feat, W):
    feat = np.asarray(feat, dtype=np.float32)
    W = np.asarray(W, dtype=np.float32)

    g = np.sign(feat) * np.sqrt(np.abs(feat))
    norm = np.sqrt(np.sum(np.abs(feat), axis=1, dtype=np.float64) ** 2
                   + EPS_SQRT * float(D) * float(D))
    norm = np.maximum(norm, EPS_NORM)

    W4 = W.reshape(C, NB, P, NB, P)  # [c, bi, i, bj, j]
    gbf = g.astype(ml_dtypes.bfloat16).astype(np.float32)
    gT = np.ascontiguousarray(gbf.T)  # [D, B]

    in_maps = []
    lams = []
    for k in range(N_CORES):
        groups = _core_cols(k)
        wk = np.empty((C, NS, P, P), dtype=np.float32)  # [c, t, i, j]
        lam = np.empty((C, 2), dtype=np.float32)
        t = 0
        for grp, (bj, bis) in enumerate(groups):
            ts = slice(t, t + len(bis))
            for dt_, bi in enumerate(bis):
                blk = W4[:, bi, :, bj, :]
                if bi != bj:
                    blk = blk + W4[:, bj, :, bi, :].transpose(0, 2, 1)
                wk[:, t + dt_] = blk
            lam[:, grp] = np.abs(wk[:, ts]).max(axis=(1, 2, 3)) / SMAX
            wk[:, ts] /= lam[:, grp][:, None, None, None]
            t += len(bis)
        lams.append(lam)
        # [c, t, i, j] -> [i, (c, t, j)]
        wk8 = (wk.transpose(2, 0, 1, 3).reshape(P, C * NS * P)
               .astype(ml_dtypes.float8_e3m4))

        gg = np.empty((P, NS * B + 8 * GW), dtype=np.float32)
        t = 0
        for bj, bis in groups:
            for bi in bis:
                gg[:, t * B:(t + 1) * B] = gT[bi * P:(bi + 1) * P]
                t += 1
        base = NS * B
        for grp, (bj, bis) in enumerate(groups):
            for cc in range(8):
                o = base + cc * GW + grp * B
                gg[:, o:o + B] = gT[bj * P:(bj + 1) * P]
        in_maps.append({
            "w": np.ascontiguousarray(wk8),
            "gg": gg.astype(ml_dtypes.bfloat16),
        })
    return in_maps, norm, np.stack(lams)  # lams [cores, C, 2]


def _run(inputs, trace=False, repeat=1):
    feat, W, b = inputs["feat"], inputs["W"], inputs["b"]
    assert feat.shape == (B, D) and W.shape == (C, D * D)

    key = ("nc", repeat)
    if key not in _CACHE:
        _CACHE[key] = _build_bass(repeat)
    nc = _CACHE[key]

    in_maps, norm, lams = _prep_inputs(feat, W)
    res = run_bass_kernel_spmd(nc, in_maps, list(range(N_CORES)), trace=trace)
    parts = np.stack([r["out"] for r in res.results]).astype(np.float64)
    parts = parts.reshape(N_CORES, C, 2, B) * lams[:, :, :, None]
    parts = parts.sum(axis=(0, 2)).T  # [B, C]
    out = parts / norm[:, None] + np.asarray(b, dtype=np.float64)[None, :]
    return out.astype(np.float32), res


def kernel(**inputs):
    return _run(inputs)[0]
